# revision 19
# baseline (speedup 1.0000x reference)
"""Trainium2 Bass kernel for nn_Block_74363063763569 (BEiT-style transformer block).

Data-parallel over batch across 8 NeuronCores (8 elems/core), zero collectives.
Self-contained: builds, compiles (cached) and runs the Bass kernel via
run_bass_kernel_spmd on cores 0-7.
"""
import sys, json
sys.path.insert(0, "/opt/trn_rl_repo")
import numpy as np


def _legalize_waits(bir_bytes, max_waits=1):
    """This container's walrus rejects >1 sync wait per instruction; split
    extras into preceding single-wait EventSemaphore instructions."""
    j = json.loads(bir_bytes)
    for f in j["functions"]:
        for b in f["blocks"]:
            out = []
            for inst in b["instructions"]:
                si = inst.get("sync_info")
                waits = si.get("on_wait", []) if si else []
                if len(waits) > max_waits:
                    keep, extra = waits[:max_waits], waits[max_waits:]
                    for k, w in enumerate(extra):
                        out.append({"debug": inst.get("debug", 0), "engine": inst["engine"],
                                    "ins": [], "name": f"{inst['name']}_w{k}",
                                    "opcode": "EventSemaphore", "outs": [],
                                    "sync_info": {"on_update": [], "on_wait": [w]}})
                    si["on_wait"] = keep
                out.append(inst)
            b["instructions"] = out
    return json.dumps(j).encode()


"""Kernel strategy (per core, data-parallel over batch; 8 elems = 4 pairs):

- Residual x0 in fp32 [tokens, C]; post-attention residual x1 in bf16.
- Matmul activations transposed [C, 2, N] bf16 via PE transposes.
- LN affine folded into qkv/fc1; gamma1/2 folded into proj/fc2; attention
  scale folded into q weights; rel-pos bias applied as exp(rpb) multiply
  on DVE (PSUM logits stay pure q.k).
- Attention per head-pair hp: row-packed logits (2 heads concurrent on PE
  via 64-row tile_position), one exp per tk-tile covering 2 heads x 2
  elems, denominator via ones-column of V + batched Ln/Exp + gpsimd
  partition_broadcast, O copied to SBUF to free PSUM early.
- fc1 weights streamed from HBM per output-block (re-layout [ob,f,k,c]).
- bufs=2 on transposed-activation tiles + bufs=3 on residual tags so the
  next pair's LN1/qkT/v matmuls keep the PE dense during softmax.
"""

import numpy as np

import concourse.bass as bass
import concourse.tile as tile
import concourse.mybir as mybir
from concourse.masks import make_identity

FP32 = mybir.dt.float32
BF16 = mybir.dt.bfloat16

B = 64
N = 197
C = 768
H = 12
D = 64
HID = 3072
NCORES = 8
BPC = B // NCORES          # 8 batch elems per core
NPAIRS_FULL = BPC // 2     # 4
KT = C // 128              # 6 k-tiles of 128 over features
KT2 = HID // 128           # 24 k-tiles over hidden
LN_EPS = 1e-5

# token tiling: 197 = 128 + 69
T_TILES = [(0, 128), (128, 69)]
# output chunks over feature dim 768 = 512 + 256
C_CHUNKS = [(0, 512), (512, 256)]

AL = mybir.AluOpType
AF = mybir.ActivationFunctionType


def build_nc(npairs=NPAIRS_FULL):
    nb = 2 * npairs
    nc = bass.Bass()

    x_d = nc.dram_tensor("x", [nb, N, C], FP32, kind="ExternalInput")
    qkvT_d = nc.dram_tensor("qkvT", [C, 3 * C], BF16, kind="ExternalInput")
    projT_d = nc.dram_tensor("projT", [C, C], BF16, kind="ExternalInput")
    fc1s_d = nc.dram_tensor("fc1s", [KT2, 128, KT, 128], BF16, kind="ExternalInput")
    fc2T_d = nc.dram_tensor("fc2T", [HID, C], BF16, kind="ExternalInput")
    erpb0_d = nc.dram_tensor("erpb0", [128, H, N], BF16, kind="ExternalInput")
    erpb1_d = nc.dram_tensor("erpb1", [69, H, N], BF16, kind="ExternalInput")
    qb_d = nc.dram_tensor("qb", [128, KT], FP32, kind="ExternalInput")
    kb_d = nc.dram_tensor("kb", [128, KT], FP32, kind="ExternalInput")
    fc1b_d = nc.dram_tensor("fc1b", [128, KT2], FP32, kind="ExternalInput")
    pb_d = nc.dram_tensor("pbrow", [C], BF16, kind="ExternalInput")
    f2b_d = nc.dram_tensor("f2brow", [C], BF16, kind="ExternalInput")
    y_d = nc.dram_tensor("y", [nb, N, C], FP32, kind="ExternalOutput")

    with tile.TileContext(nc) as tc:
        with (
            tc.tile_pool(name="singles", bufs=1) as singles,
            tc.tile_pool(name="resid", bufs=3) as resid,     # x0 + out (fp32), rotating
            tc.tile_pool(name="x1p", bufs=1) as x1p,         # x1 bf16
            tc.tile_pool(name="b394", bufs=2) as b394,       # bf16 [128, 2, N] transposed acts
            tc.tile_pool(name="xn", bufs=1) as xnp,
            tc.tile_pool(name="vpool", bufs=2) as vpool,
            tc.tile_pool(name="fc1sp", bufs=3) as fc1sp,
            tc.tile_pool(name="etp", bufs=2) as etp,
            tc.tile_pool(name="ocpp", bufs=2) as ocpp,
            tc.tile_pool(name="small", bufs=8) as small,
            tc.tile_pool(name="ps_mm", bufs=2, space="PSUM") as ps_mm,
            tc.tile_pool(name="ps_l0", bufs=1, space="PSUM") as ps_l0,
            tc.tile_pool(name="ps_l1", bufs=1, space="PSUM") as ps_l1,
            tc.tile_pool(name="ps_o", bufs=1, space="PSUM") as ps_o,
        ):
            # ---- persistent weights / constants ----
            qkvT = [singles.tile([128, 3 * C], BF16, tag=f"qkvT{k}", name=f"qkvT{k}") for k in range(KT)]
            projT = [singles.tile([128, C], BF16, tag=f"projT{k}", name=f"projT{k}") for k in range(KT)]
            fc2T = [singles.tile([128, C], BF16, tag=f"fc2T{k}", name=f"fc2T{k}") for k in range(KT2)]
            erpb0 = singles.tile([128, H, N], BF16, tag="erpb0")
            erpb1 = singles.tile([69, H, N], BF16, tag="erpb1")
            qb_sb = singles.tile([128, KT], FP32, tag="qb")
            kb_sb = singles.tile([128, KT], FP32, tag="kb")
            fc1b_sb = singles.tile([128, KT2], FP32, tag="fc1b")
            brow3 = singles.tile([65, C], BF16, tag="brow3")
            pb_sb = brow3[32:33, :]
            f2b_sb = brow3[64:65, :]
            ident = singles.tile([128, 128], BF16, tag="ident")
            ones_col = singles.tile([65, 128], BF16, tag="ones")
            eps_sb = singles.tile([128, 1], FP32, tag="eps")

            for k in range(3):
                nc.sync.dma_start(qkvT[k][:], qkvT_d[k * 128:(k + 1) * 128, :])
            for k in range(3, KT):
                nc.scalar.dma_start(qkvT[k][:], qkvT_d[k * 128:(k + 1) * 128, :])
            for k in range(KT):
                nc.gpsimd.dma_start(projT[k][:], projT_d[k * 128:(k + 1) * 128, :])
            nc.scalar.dma_start(erpb0[:], erpb0_d[:])
            nc.scalar.dma_start(erpb1[:], erpb1_d[:])
            for k in range(KT2):
                nc.scalar.dma_start(fc2T[k][:], fc2T_d[k * 128:(k + 1) * 128, :])
            nc.scalar.dma_start(qb_sb[:], qb_d[:, :])
            nc.scalar.dma_start(kb_sb[:], kb_d[:, :])
            nc.scalar.dma_start(fc1b_sb[:], fc1b_d[:, :])
            nc.scalar.dma_start(brow3[32:33, :], pb_d[None, :])
            nc.scalar.dma_start(brow3[64:65, :], f2b_d[None, :])
            make_identity(nc, ident[:])
            nc.vector.memset(ones_col[:], 1.0)
            nc.vector.memset(eps_sb[:], LN_EPS)

            def ln_transpose(x_tiles, tag, out_tags, tile_bufs=None):
                """LN over feature dim + PE-transpose into [128, 2, N] bf16 tiles."""
                xT = [b394.tile([128, 2, N], BF16, tag=out_tags[k], name=f"{tag}T{k}", bufs=tile_bufs)
                      for k in range(KT)]
                for (e, j), xt in x_tiles.items():
                    toff, tcnt = T_TILES[j]
                    stats = small.tile([128, 3, 6], FP32, tag=f"st_{tag}", bufs=4)
                    mv = small.tile([128, 2], FP32, tag=f"mv_{tag}")
                    sd = small.tile([128, 1], FP32, tag=f"sd_{tag}")
                    rstd = small.tile([128, 1], FP32, tag=f"rs_{tag}")
                    for g in range(3):
                        nc.vector.bn_stats(stats[:tcnt, g, :], xt[:tcnt, g * 256:(g + 1) * 256])
                    nc.vector.bn_aggr(mv[:tcnt], stats[:tcnt])
                    nc.scalar.activation(sd[:tcnt], mv[:tcnt, 1:2], AF.Ln, bias=eps_sb[:tcnt])
                    nc.scalar.activation(rstd[:tcnt], sd[:tcnt], AF.Exp, scale=-0.5)
                    xn = xnp.tile([128, C], BF16, tag="xn")
                    nc.vector.tensor_scalar(
                        xn[:tcnt, :], xt[:tcnt, :],
                        mv[:tcnt, 0:1], rstd[:tcnt, 0:1],
                        op0=AL.subtract, op1=AL.mult)
                    for cb in range(KT):
                        pt = ps_mm.tile([128, 512], BF16, tag="mm", name=f"tr_{tag}")
                        nc.tensor.transpose(
                            pt[:128, :tcnt],
                            xn[:tcnt, cb * 128:(cb + 1) * 128],
                            ident[:tcnt, :tcnt])
                        nc.vector.tensor_copy(
                            xT[cb][:, e, toff:toff + tcnt],
                            pt[:128, :tcnt])
                return xT

            def ln1_unit(fs, e, j):
                xt = fs['x0'][(e, j)]
                xT = fs['xnT']
                toff, tcnt = T_TILES[j]
                stats = small.tile([128, 3, 6], FP32, tag="st_ln1", bufs=4)
                mv = small.tile([128, 2], FP32, tag="mv_ln1")
                sd = small.tile([128, 1], FP32, tag="sd_ln1")
                rstd = small.tile([128, 1], FP32, tag="rs_ln1")
                for g in range(3):
                    nc.vector.bn_stats(stats[:tcnt, g, :], xt[:tcnt, g * 256:(g + 1) * 256])
                nc.vector.bn_aggr(mv[:tcnt], stats[:tcnt])
                nc.scalar.activation(sd[:tcnt], mv[:tcnt, 1:2], AF.Ln, bias=eps_sb[:tcnt])
                nc.scalar.activation(rstd[:tcnt], sd[:tcnt], AF.Exp, scale=-0.5)
                xn = xnp.tile([128, C], BF16, tag="xn")
                nc.vector.tensor_scalar(
                    xn[:tcnt, :], xt[:tcnt, :],
                    mv[:tcnt, 0:1], rstd[:tcnt, 0:1],
                    op0=AL.subtract, op1=AL.mult)
                for cb in range(KT):
                    pt = ps_mm.tile([128, 512], BF16, tag="mm", name="tr_ln1")
                    nc.tensor.transpose(
                        pt[:128, :tcnt],
                        xn[:tcnt, cb * 128:(cb + 1) * 128],
                        ident[:tcnt, :tcnt])
                    nc.vector.tensor_copy(
                        xT[cb][:, e, toff:toff + tcnt],
                        pt[:128, :tcnt])

            def qk_obs(fs, which, obs):
                dst = fs['qT'] if which == 'q' else fs['kT']
                base = 0 if which == 'q' else C
                bias = qb_sb if which == 'q' else kb_sb
                xnT = fs['xnT']
                for ob in obs:
                    ps = ps_mm.tile([128, 2, N], FP32, tag="mm")
                    for k in range(KT):
                        nc.tensor.matmul(
                            ps[:, :, :], qkvT[k][:, base + ob * 128: base + (ob + 1) * 128],
                            xnT[k][:, :, :], start=(k == 0), stop=(k == KT - 1))
                    nc.vector.tensor_scalar_add(dst[ob][:, :, :], ps[:, :, :], bias[:, ob:ob + 1])

            def v_unit(fs, e, j):
                toff, tcnt = T_TILES[j]
                xnT = fs['xnT']
                vt = vpool.tile([128, H, D + 1], BF16, tag=f"v{e}{j}", bufs=2 if e == 0 else 1)
                nc.vector.memset(vt[:, :, D:D + 1], 1.0)
                for ci, (coff, csz) in enumerate(C_CHUNKS):
                    ps = ps_mm.tile([128, 512], FP32, tag="mm")
                    for k in range(KT):
                        nc.tensor.matmul(
                            ps[:tcnt, :csz],
                            xnT[k][:, e, toff:toff + tcnt],
                            qkvT[k][:, 2 * C + coff: 2 * C + coff + csz],
                            start=(k == 0), stop=(k == KT - 1))
                    h0 = coff // D
                    nh = csz // D
                    nc.vector.tensor_copy(
                        vt[:tcnt, h0:h0 + nh, 0:D],
                        ps[:tcnt, :csz])
                fs['v'][(e, j)] = vt

            def make_front(s):
                fs = {'v': {}}

                def c0():
                    fs['x0'] = {}
                    for e in range(2):
                        bidx = 2 * s + e
                        for j, (toff, tcnt) in enumerate(T_TILES):
                            t = resid.tile([128, C], FP32, tag=f"x0_{e}{j}", name=f"x0_{e}{j}_{s}")
                            nc.scalar.dma_start(t[:tcnt, :], x_d[bidx, toff:toff + tcnt, :])
                            fs['x0'][(e, j)] = t
                    fs['xnT'] = [b394.tile([128, 2, N], BF16, tag=f"b394_xnT{k}", name=f"xnT{k}_{s}")
                                 for k in range(KT)]
                    ln1_unit(fs, 0, 0)

                def c1():
                    ln1_unit(fs, 0, 1)
                    ln1_unit(fs, 1, 0)

                def c2():
                    ln1_unit(fs, 1, 1)
                    fs['qT'] = [b394.tile([128, 2, N], BF16, tag=f"b394_qT{ob}", name=f"qT{ob}_{s}")
                                for ob in range(KT)]
                    fs['kT'] = [b394.tile([128, 2, N], BF16, tag=f"b394_kT{ob}", name=f"kT{ob}_{s}")
                                for ob in range(KT)]

                def c3():
                    qk_obs(fs, 'q', range(KT))

                def c4():
                    qk_obs(fs, 'k', range(KT))

                def c5():
                    for e in range(2):
                        for j in range(2):
                            v_unit(fs, e, j)

                return fs, [c0, c1, c2, c3, c4, c5]

            def attn_hp(fs, s, hp):
                qT, kT, v_sb, aT = fs['qT'], fs['kT'], fs['v'], fs['aT']
                hA = 2 * hp
                Lj0 = ps_l0.tile([128, 4, 256], FP32, tag="Lj0")
                Lj1 = ps_l1.tile([69, 4, 256], FP32, tag="Lj1")
                for e in range(2):
                    for jt, Lt, (tkoff, tkcnt) in ((0, Lj0, T_TILES[0]), (1, Lj1, T_TILES[1])):
                        for hl in range(2):
                            rbase = 64 * hl
                            sl = 2 * hl + e
                            nc.tensor.matmul(
                                Lt[:tkcnt, sl, 0:N],
                                kT[hp][rbase:rbase + 64, e, tkoff:tkoff + tkcnt],
                                qT[hp][rbase:rbase + 64, e, :],
                                start=True, stop=True)
                et0 = etp.tile([128, 4, N], BF16, tag="et0")
                et1 = etp.tile([69, 4, N], BF16, tag="et1")
                nc.scalar.activation(et0[:, :, :], Lj0[:, :, 0:N], AF.Exp)
                nc.scalar.activation(et1[:69, :, :], Lj1[:69, :, 0:N], AF.Exp)
                for hl in range(2):
                    h = hA + hl
                    sl = slice(2 * hl, 2 * hl + 2)
                    nc.vector.tensor_tensor(
                        et0[:, sl, :], et0[:, sl, :],
                        erpb0[:, h:h + 1, :].broadcast_to([128, 2, N]), op=AL.mult)
                    nc.vector.tensor_tensor(
                        et1[:69, sl, :], et1[:69, sl, :],
                        erpb1[:69, h:h + 1, :].broadcast_to([69, 2, N]), op=AL.mult)
                O = ps_o.tile([65, 4, 256], FP32, tag="O")
                for e in range(2):
                    for hl in range(2):
                        h = hA + hl
                        sl = 2 * hl + e
                        nc.tensor.matmul(
                            O[:65, sl, 0:N],
                            v_sb[(e, 0)][:128, h, :],
                            et0[:128, sl, :], start=True, stop=False)
                        nc.tensor.matmul(
                            O[:65, sl, 0:N],
                            v_sb[(e, 1)][:69, h, :],
                            et1[:69, sl, :], start=False, stop=True)
                ocp = ocpp.tile([65, 4, N], BF16, tag="ocp")
                nc.vector.tensor_copy(ocp[:, :, :], O[:65, :, 0:N])
                lden = small.tile([1, 4, N], BF16, tag="lden", bufs=2)
                nc.scalar.activation(lden[:, :, :], ocp[64:65, :, :], AF.Ln)
                nc.scalar.activation(lden[:, :, :], lden[:, :, :], AF.Exp, scale=-1.0)
                rbn = ps_o.tile([64, 4, 256], FP32, tag="O", name="rbn")
                for hl in range(2):
                    nc.tensor.matmul(
                        rbn[0:64, 2 * hl:2 * hl + 2, 0:N],
                        ones_col[0:1, 0:64],
                        lden[0:1, 2 * hl:2 * hl + 2, :],
                        start=True, stop=True)
                for hl in range(2):
                    rbase = 64 * hl
                    sl = slice(2 * hl, 2 * hl + 2)
                    nc.vector.tensor_tensor(
                        aT[hp][rbase:rbase + 64, :, :],
                        ocp[0:64, sl, :], rbn[0:64, sl, 0:N], op=AL.mult)

            def make_back(fs, s):
                st = {}
                aT, x0 = fs['aT'], fs['x0']

                def proj_unit(e, j):
                    toff, tcnt = T_TILES[j]
                    xt = st['x1'][(e, j)]
                    for ci, (coff, csz) in enumerate(C_CHUNKS):
                        ps = ps_mm.tile([128, 512], FP32, tag="mm")
                        for k in range(KT):
                            nc.tensor.matmul(
                                ps[:tcnt, :csz],
                                aT[k][:, e, toff:toff + tcnt],
                                projT[k][:, coff:coff + csz],
                                start=(k == 0), stop=False)
                        nc.tensor.matmul(
                            ps[:tcnt, :csz],
                            ones_col[32:33, :tcnt],
                            pb_sb[:, coff:coff + csz],
                            start=False, stop=True)
                        nc.vector.tensor_tensor(
                            xt[:tcnt, coff:coff + csz],
                            ps[:tcnt, :csz],
                            x0[(e, j)][:tcnt, coff:coff + csz], op=AL.add)

                def fc1_obs(obs):
                    hnT = st['hnT']
                    for ob in obs:
                        fst = fc1sp.tile([128, KT, 128], BF16, tag="fc1s")
                        nc.sync.dma_start(fst[:, :, :], fc1s_d[ob])
                        ps = ps_mm.tile([128, 2, N], FP32, tag="mm")
                        for k in range(KT):
                            nc.tensor.matmul(
                                ps[:, :, :], fst[:, k, :],
                                hnT[k][:, :, :], start=(k == 0), stop=(k == KT - 1))
                        nc.scalar.activation(
                            st['hT'][ob][:, :, :], ps[:, :, :], AF.Gelu,
                            bias=fc1b_sb[:, ob:ob + 1])

                def fc2_unit(e, j):
                    bidx = 2 * s + e
                    toff, tcnt = T_TILES[j]
                    hT = st['hT']
                    ot = resid.tile([128, C], FP32, tag=f"x0_{e}{j}", name=f"out_{e}{j}_{s}")
                    for ci, (coff, csz) in enumerate(C_CHUNKS):
                        ps = ps_mm.tile([128, 512], FP32, tag="mm")
                        for k in range(KT2):
                            nc.tensor.matmul(
                                ps[:tcnt, :csz],
                                hT[k][:, e, toff:toff + tcnt],
                                fc2T[k][:, coff:coff + csz],
                                start=(k == 0), stop=False)
                        nc.tensor.matmul(
                            ps[:tcnt, :csz],
                            ones_col[64:65, :tcnt],
                            f2b_sb[:, coff:coff + csz],
                            start=False, stop=True)
                        nc.vector.tensor_tensor(
                            ot[:tcnt, coff:coff + csz],
                            ps[:tcnt, :csz],
                            st['x1'][(e, j)][:tcnt, coff:coff + csz], op=AL.add)
                    nc.gpsimd.dma_start(y_d[bidx, toff:toff + tcnt, :], ot[:tcnt, :])

                def main_part():
                    st['x1'] = {(e, j): x1p.tile([128, C], BF16, tag=f"x1_{e}{j}", name=f"x1_{e}{j}_{s}")
                                for e in range(2) for j in range(2)}
                    proj_unit(0, 0)
                    proj_unit(0, 1)
                    proj_unit(1, 0)
                    proj_unit(1, 1)
                    st['hnT'] = ln_transpose(st['x1'], "ln2", [f"b394_hnT{k}" for k in range(KT)], tile_bufs=1)
                    _ht_tags = ([f"b394_xnT{k}" for k in range(KT)] + [f"b394_h2{k}" for k in range(KT)]
                                + [f"b394_h{k}" for k in range(KT)] + [f"b394_aT{k}" for k in range(KT)])
                    _ht_bufs = [2] * KT + [1] * KT + [1] * KT + [2] * KT
                    st['hT'] = [b394.tile([128, 2, N], BF16, tag=_ht_tags[ob], name=f"hT{ob}_{s}",
                                          bufs=_ht_bufs[ob])
                                for ob in range(KT2)]
                    fc1_obs(range(0, 24))

                main_part()
                fc2_unit(0, 0)
                fc2_unit(0, 1)
                fc2_unit(1, 0)
                fc2_unit(1, 1)

            # ---- software-pipelined main loop ----
            # window s: attn_hp(s) | front_chunk(s+1); then back(s)
            fs0, chunks0 = make_front(0)
            for c in chunks0:
                c()
            fronts = {0: fs0}
            for s in range(npairs):
                fs = fronts[s]
                fs['aT'] = [b394.tile([128, 2, N], BF16, tag=f"b394_aT{cb}", name=f"aT{cb}_{s}")
                            for cb in range(KT)]
                nchunks = None
                if s + 1 < npairs:
                    fronts[s + 1], nchunks = make_front(s + 1)
                for hp in range(KT):
                    attn_hp(fs, s, hp)
                    if nchunks is not None:
                        nchunks[hp]()
                make_back(fs, s)
                del fronts[s]
    return nc


def fold_weights(inputs):
    """Host-side folding. Returns dict of per-core-shared input arrays."""
    import ml_dtypes
    f32 = np.float32
    bf16 = ml_dtypes.bfloat16
    g = {k: np.asarray(v) for k, v in inputs.items()}
    n1w, n1b = g["n1_w"].astype(f32), g["n1_b"].astype(f32)
    n2w, n2b = g["n2_w"].astype(f32), g["n2_b"].astype(f32)
    g1, g2 = g["gamma1"].astype(f32), g["gamma2"].astype(f32)
    qkv_w = g["qkv_w"].astype(f32)
    q_bias, v_bias = g["q_bias"].astype(f32), g["v_bias"].astype(f32)
    proj_w, proj_b = g["proj_w"].astype(f32), g["proj_b"].astype(f32)
    fc1_w, fc1_b = g["fc1_w"].astype(f32), g["fc1_b"].astype(f32)
    fc2_w, fc2_b = g["fc2_w"].astype(f32), g["fc2_b"].astype(f32)

    qkv_bias = np.concatenate([q_bias, np.zeros_like(q_bias), v_bias])
    Wq = qkv_w * n1w[None, :]
    bq = qkv_bias + qkv_w @ n1b
    bq_v_placeholder = bq[2 * C:]
    scale = (C // H) ** -0.5
    Wq[:C] *= scale
    bq[:C] *= scale

    Pw = g1[:, None] * proj_w
    pb = g1 * (proj_b + proj_w @ bq_v_placeholder)
    F1 = fc1_w * n2w[None, :]
    f1b = fc1_b + fc1_w @ n2b
    F2 = g2[:, None] * fc2_w
    f2b = g2 * fc2_b

    table = g["rel_bias_table"].astype(f32)
    idx = np.asarray(g["rel_index"]).reshape(-1)
    rpb_ref = table[idx].reshape(N, N, H).transpose(2, 0, 1)   # [h, tq, tk]
    erpbT = np.exp(rpb_ref.transpose(0, 2, 1))                 # [h, tk, tq]
    erpb0 = np.ascontiguousarray(erpbT[:, :128, :].transpose(1, 0, 2)).astype(bf16)
    erpb1 = np.ascontiguousarray(erpbT[:, 128:, :].transpose(1, 0, 2)).astype(bf16)

    F1T = np.ascontiguousarray(F1.T)                           # [C, HID]
    fc1s = np.ascontiguousarray(
        F1T.reshape(KT, 128, KT2, 128).transpose(2, 1, 0, 3)).astype(bf16)

    return {
        "qkvT": np.ascontiguousarray(Wq.T).astype(bf16),
        "projT": np.ascontiguousarray(Pw.T).astype(bf16),
        "fc1s": fc1s,
        "fc2T": np.ascontiguousarray(F2.T).astype(bf16),
        "erpb0": erpb0,
        "erpb1": erpb1,
        "qb": np.ascontiguousarray(bq[:C].reshape(KT, 128).T),
        "kb": np.ascontiguousarray(bq[C:2 * C].reshape(KT, 128).T),
        "fc1b": np.ascontiguousarray(f1b.reshape(KT2, 128).T),
        "pbrow": pb.astype(bf16),
        "f2brow": f2b.astype(bf16),
    }


_CACHE = {}


def _get_nc():
    if "nc" not in _CACHE:
        nc = build_nc()
        patched = _legalize_waits(nc.to_json_bytes())
        nc.to_json_bytes = lambda: patched
        _CACHE["nc"] = nc
    return _CACHE["nc"]


_LDW_PATCHED = []


def _enable_ldw_opt():
    """Rewrite the walrus argv to enable LDWEIGHTS scheduling optimization
    (hidden weight loads). Disable with BASS_NO_LDW_OPT=1."""
    import os
    if not os.environ.get("BASS_LDW_OPT"):
        return
    if _LDW_PATCHED:
        return
    import concourse.bass_utils as bu
    orig = bu.run_command

    def patched(argv, **kw):
        if isinstance(argv, (list, tuple)) and any("walrus_driver" in str(a) for a in argv):
            argv = ["--enable-ldw-opt=true" if str(a) == "--enable-ldw-opt=false" else a
                    for a in argv]
        return orig(argv, **kw)

    bu.run_command = patched
    _LDW_PATCHED.append(True)


def kernel(**inputs):
    from concourse.bass_utils import run_bass_kernel_spmd
    _enable_ldw_opt()
    nc = _get_nc()
    folded = fold_weights(inputs)
    x = np.ascontiguousarray(np.asarray(inputs["x"], dtype=np.float32))
    assert x.shape == (B, N, C), x.shape
    in_maps = []
    for c in range(NCORES):
        m = dict(folded)
        m["x"] = np.ascontiguousarray(x[c * BPC:(c + 1) * BPC])
        in_maps.append(m)
    res = run_bass_kernel_spmd(nc, in_maps, core_ids=list(range(NCORES)))
    out = np.concatenate([res.results[c]["y"] for c in range(NCORES)], axis=0)
    return out.astype(np.float32)
